# revision 5
# baseline (speedup 1.0000x reference)
"""CTRNN Bass/Tile kernel for Trainium2, 8-core batch-parallel SPMD.

Math (per core, batch shard B_s=8):
  reference:  h_{t+1} = (1-a) h_t + a*sigmoid(xp_t + h_t @ W_h.T),  a = 1/50
  substitution g = h/a, What = a*W_h (folded host-side):
      z_t = xp_t + g_t @ What.T
      g_{t+1} = (1-a) g_t + sigmoid(z_t)
      output[t] = h_{t+1} = a * g_{t+1}   (the a-scale applied host-side)

Device layout: state g kept transposed as [128(h_sub), 4(h_chunk), 8(batch)].
Per step: 16 matmuls (4 k-chunks x 4 m-chunks, bf16 weights stationary),
sigmoid on ACT in two half-groups (chunks {0,1} / {2,3}), blend on DVE,
bf16 cast of next-step rhs on GPSIMD.  xp = x @ W_in.T precomputed on-device
into SBUF (bf16).  Outputs staged 64 steps at a time, DMA'd as [4,128,8,T],
transposed back to [T,B,H] on host.
"""

import os
import sys
import time

import numpy as np

for _p in ("/opt/trn_rl_repo",):
    if os.path.isdir(_p) and _p not in sys.path:
        sys.path.append(_p)

import ml_dtypes

import concourse.bacc as bacc
import concourse.bass as bass
import concourse.mybir as mybir
import concourse.tile as tile
from concourse import bass_utils

ALPHA = 1.0 / 50.0
DECAY = float(np.float32(1.0) - np.float32(1.0 / 50.0))

T, B, I, H = 1024, 64, 128, 512
NCORES = 8
BS = B // NCORES  # 8 batch rows per core
NCH = H // 128  # 4 h-chunks
BLK = 64  # output staging block (steps per out DMA)

F32 = mybir.dt.float32
BF16 = mybir.dt.bfloat16
SIG = mybir.ActivationFunctionType.Sigmoid

# module-level cache: (T,) -> (nc, meta)
_BUILT = {}
LAST_RESULTS = None  # BassKernelResults from the most recent run (for test.py)


def _build(t_steps: int):
    """Build the Bass module for a t_steps-long scan. Returns compiled nc."""
    if t_steps in _BUILT:
        return _BUILT[t_steps]

    n = t_steps * BS  # columns of the xp GEMM per core
    blk = min(BLK, t_steps)
    assert t_steps % blk == 0

    nc = bacc.Bacc("TRN2", target_bir_lowering=False, debug=False)

    xT_d = nc.dram_tensor("xT", [I, n], BF16, kind="ExternalInput").ap()
    wt_d = nc.dram_tensor("Wt", [128, NCH * NCH * 128], BF16, kind="ExternalInput").ap()
    win_d = nc.dram_tensor("WinT", [I, H], BF16, kind="ExternalInput").ap()
    g0_d = nc.dram_tensor("g0", [128, NCH * BS], F32, kind="ExternalInput").ap()
    out_d = nc.dram_tensor("outT", [NCH, 128, BS, t_steps], F32, kind="ExternalOutput").ap()
    out_v = out_d.rearrange("c h b t -> h c b t")

    with tile.TileContext(nc) as tc:
        with (
            tc.tile_pool(name="const", bufs=1) as cpool,
            tc.tile_pool(name="stage", bufs=2) as stage_pool,
            tc.tile_pool(name="state", bufs=3) as spool,
            tc.tile_pool(name="psum", bufs=3, space="PSUM") as ppool,
        ):
            # ---- resident constants ----
            w_sb = cpool.tile([128, NCH * NCH * 128], BF16)  # (k_sub,(k,m,m_sub))
            nc.sync.dma_start(w_sb[:], wt_d[:])
            win_sb = cpool.tile([I, H], BF16)
            nc.sync.dma_start(win_sb[:], win_d[:])
            g0_sb = cpool.tile([128, NCH, BS], F32)
            nc.sync.dma_start(g0_sb[:], g0_d.rearrange("h (c b) -> h c b", c=NCH))
            xt_sb = cpool.tile([I, n], BF16)
            nc.sync.dma_start(xt_sb[:], xT_d[:])
            zb = cpool.tile([128, 1], F32)
            nc.vector.memset(zb[:], 0.0)

            # ---- xp = x @ W_in.T, transposed into SBUF: [128, c, t*BS] bf16 ----
            xp_sb = cpool.tile([128, NCH, n], BF16)
            with tc.tile_pool(name="xpp", bufs=2, space="PSUM") as xp_psum:
                n_t = 512  # free-dim per matmul
                copy_rr = 0
                for c in range(NCH):
                    for j in range(0, n, n_t):
                        w = min(n_t, n - j)
                        ps = xp_psum.tile([128, n_t], F32, tag="xps")
                        nc.tensor.matmul(
                            ps[:, :w],
                            win_sb[:, c * 128 : (c + 1) * 128],
                            xt_sb[:, j : j + w],
                            start=True,
                            stop=True,
                        )
                        # rotate PSUM->SBUF bf16 copies between DVE and ACT
                        if copy_rr % 2 == 0:
                            nc.vector.tensor_copy(xp_sb[:, c, j : j + w], ps[:, :w])
                        else:
                            nc.scalar.activation(
                                xp_sb[:, c, j : j + w], ps[:, :w],
                                mybir.ActivationFunctionType.Copy,
                            )
                        copy_rr += 1

            # ---- initial state ----
            gb_cur = [spool.tile([128, 2, BS], BF16, tag=f"gb{g}", name=f"gb{g}") for g in range(2)]
            gt_cur = [spool.tile([128, 2, BS], F32, tag=f"gt{g}", name=f"gt{g}") for g in range(2)]
            for g in range(2):
                cr = slice(2 * g, 2 * g + 2)
                nc.gpsimd.tensor_copy(gb_cur[g][:], g0_sb[:, cr, :])
                nc.vector.tensor_scalar_mul(gt_cur[g][:], g0_sb[:, cr, :], DECAY)

            # ---- the scan ----
            stage = None
            for t in range(t_steps):
                tt = t % blk
                if tt == 0:
                    stage = stage_pool.tile([128, NCH, BS, blk], F32, tag="stage")

                psA = ppool.tile([128, 2, BS], F32, tag="psA")
                psB = ppool.tile([128, 2, BS], F32, tag="psB")
                ps_of = {0: (psA, 0), 1: (psA, 1), 2: (psB, 0), 3: (psB, 1)}

                # PE: phase k in {0,1} (needs gb_cur[0]), then k in {2,3}
                first_in_bank = {id(psA): True, id(psB): True}
                for kg in range(2):
                    for m in range(NCH):
                        ps, mloc = ps_of[m]
                        for k in (2 * kg, 2 * kg + 1):
                            st = first_in_bank[id(ps)]
                            first_in_bank[id(ps)] = False
                            last = kg == 1 and mloc == 1 and k == 2 * kg + 1
                            nc.tensor.matmul(
                                ps[:, mloc, :],
                                w_sb[:, (k * NCH + m) * 128 : (k * NCH + m + 1) * 128],
                                gb_cur[kg][:, k - 2 * kg, :],
                                start=st,
                                stop=last,
                                skip_group_check=True,
                            )

                gb_nxt = [spool.tile([128, 2, BS], BF16, tag=f"gb{g}", name=f"gb{g}") for g in range(2)]
                gt_nxt = [spool.tile([128, 2, BS], F32, tag=f"gt{g}", name=f"gt{g}") for g in range(2)]
                for g, ps in ((0, psA), (1, psB)):
                    cr = slice(2 * g, 2 * g + 2)
                    z = spool.tile([128, 2, BS], F32, tag=f"z{g}", name=f"z{g}")
                    nc.vector.tensor_add(z[:], ps[:], xp_sb[:, cr, t * BS : (t + 1) * BS])
                    s = spool.tile([128, 2, BS], F32, tag=f"s{g}", name=f"s{g}")
                    nc.scalar.activation(s[:], z[:], SIG, bias=zb[:, 0:1])
                    gs = stage[:, cr, :, tt]
                    nc.vector.tensor_add(gs, gt_cur[g][:], s[:])
                    nc.gpsimd.tensor_copy(gb_nxt[g][:], gs)
                    nc.vector.tensor_scalar_mul(gt_nxt[g][:], gs, DECAY)
                gb_cur, gt_cur = gb_nxt, gt_nxt

                if tt == blk - 1:
                    for c in range(NCH):
                        nc.sync.dma_start(
                            out_v[:, c, :, t - blk + 1 : t + 1], stage[:, c, :, :]
                        )

    nc.compile()
    _BUILT[t_steps] = nc
    return nc


def _prep_inputs(x, W_in, W_h, h0, t_steps):
    """Host-side shard + layout prep. Returns list of in_maps (one per core)."""
    bf = ml_dtypes.bfloat16
    what_t = (ALPHA * W_h.astype(np.float32)).T  # [512(k), 512(m)]
    wt = np.ascontiguousarray(
        what_t.reshape(NCH, 128, NCH, 128).transpose(1, 0, 2, 3).reshape(128, -1)
    ).astype(bf)
    win_t = np.ascontiguousarray(W_in.astype(np.float32).T).astype(bf)  # [I, H]
    g0 = (h0[0].astype(np.float32) / ALPHA).reshape(NCH, 128).T  # [128, c]
    g0 = np.ascontiguousarray(
        np.broadcast_to(g0[:, :, None], (128, NCH, BS)).reshape(128, NCH * BS)
    ).astype(np.float32)

    in_maps = []
    for ci in range(NCORES):
        xs = x[:t_steps, ci * BS : (ci + 1) * BS, :].astype(np.float32)
        xT = np.ascontiguousarray(xs.transpose(2, 0, 1).reshape(I, -1)).astype(bf)
        in_maps.append({"xT": xT, "Wt": wt, "WinT": win_t, "g0": g0})
    return in_maps


def _run(x, W_in, W_h, h0, t_steps, trace=False):
    global LAST_RESULTS
    nc = _build(t_steps)
    in_maps = _prep_inputs(x, W_in, W_h, h0, t_steps)
    res = bass_utils.run_bass_kernel_spmd(
        nc, in_maps, core_ids=list(range(NCORES)), trace=trace
    )
    LAST_RESULTS = res
    # assemble: per-core outT [NCH,128,BS,t] -> [t, BS, H]; concat batch; scale
    outs = []
    for ci in range(NCORES):
        o = res.results[ci]["outT"]  # [4,128,8,t]
        o = np.ascontiguousarray(o.transpose(3, 2, 0, 1)).reshape(t_steps, BS, H)
        outs.append(o)
    g = np.concatenate([o[:, None] for o in outs], axis=1)  # [t, ncores, BS, H]
    g = g.reshape(t_steps, B, H)
    out = (ALPHA * g).astype(np.float32)
    return out, out[-1].copy()


def kernel(x, W_in, W_h, h0):
    x = np.asarray(x)
    W_in = np.asarray(W_in)
    W_h = np.asarray(W_h)
    h0 = np.asarray(h0)
    return _run(x, W_in, W_h, h0, T, trace=bool(int(os.environ.get("KTRACE", "0"))))
